# revision 33
# baseline (speedup 1.0000x reference)
"""Trainium2 Bass kernel: BinaryHungarianMatcherV2 cost-matrix build.

C[b,q,t] = 5*L1(pred_box, tgt_box) + 2*focal_class(q) + 2*(-giou),
masked to 1e9 where t >= num_boxes[b].

Sharding: batch dim (32) split across 8 NeuronCores (pure data parallel,
4 batch slots per core). Per core each [Q=1800, T=500] cost slab is built
as 15 q-tiles of 128 rows (the last tile overlaps and stores only its new
8 rows), q on the partition axis, t on the free axis.

All pairwise work runs on the DVE as fused custom ops (min/max corner
diffs, relu-product, abs-sums, one reciprocal via the common denominator
union*area_e). Separable terms (corners, areas, focal class cost, the
validity mask) are precomputed on host in fp64 and shipped as per-q
scalar columns plus per-t rows replicated across partitions.

Because the 8 cores share one SPMD program, per-batch valid-column
counts are handled by sorting the 32 batches by num_boxes and assigning
similar sizes to the same program slot: slot j computes only
W[j] = max over its 8 cores of num_boxes columns (~62% of full work for
uniform sizes); columns beyond W[j] are filled with 1e9 by plain DMAs
from a constant tile.
"""

import os

import numpy as np

B, Q, T = 32, 1800, 500
N_CORES = 8
B_PER = B // N_CORES          # 4 batch slots per core
QT = 128                      # q-tile partition size (full partitions)
NQT = 15                      # 14 full tiles + 1 overlapping tile per batch

# replicated per-target rows (broadcast across partitions)
R_X0, R_Y0, R_X1, R_Y1, R_CX, R_CY, R_W, R_H, R_A2, R_MK, R_FILL = range(11)
NREP = 11
# per-query scalar columns
(S_X0, S_Y0, S_X1, S_Y1, S_CX, S_CY, S_W, S_H, S_A1, S_CC,
 S_NCX, S_NCY, S_NW, S_NH, S_CC5) = range(15)
NSC = 15

INVALID = 1.0e9

_OPS = None
_PROG_CACHE = {}
LAST_RESULTS = None


def _get_ops():
    """Register the fused custom-DVE ops (idempotent). Returns dict name->DveOp."""
    global _OPS
    if _OPS is not None:
        return _OPS
    from concourse import dve_ops
    from concourse.dve_ops import DveOp
    from concourse.dve_spec import (
        Spec, Src0, Src1, C0, C1, C2, Zero, relu, maxx, minn, select, lower,
    )
    from concourse.dve_uop import DveOpSpec

    def reg(name, spec):
        for op in dve_ops.OPS:
            if op.name == name:
                return op
        row = max(dve_ops._SUB_OPCODE_FOR_NAME.values()) + 1
        assert row < 0x20, "custom-DVE opcode rows exhausted"
        dve_ops._SUB_OPCODE_FOR_NAME[name] = row
        shas = {}
        for ver in ("v3", "v4"):
            s = DveOpSpec(name=name, opcode=row, uops=lower(spec, ver=ver),
                          rd1_en=dve_ops.has_src1(spec))
            shas[ver] = s.sha(ver)
        op = DveOp(name, spec, subdim=False, uops_sha=shas)
        dve_ops.OPS.append(op)
        dve_ops.CUSTOM_DVE_SPECS[name] = spec
        return op

    _d0 = Src0 - C0
    _d1 = Src1 - C1
    _OPS = {
        # wd = min(x1_t, x1_q) - max(x0_t, x0_q)
        "BHM_IDIFF": reg("BHM_IDIFF", Spec(
            body=minn(Src0, C0) - maxx(Src1, C1),
            reference=lambda in0, in1, s0, s1, imm2:
                np.minimum(in0, s0) - np.maximum(in1, s1))),
        # we = max(x1_t, x1_q) - min(x0_t, x0_q)
        "BHM_EDIFF": reg("BHM_EDIFF", Spec(
            body=maxx(Src0, C0) - minn(Src1, C1),
            reference=lambda in0, in1, s0, s1, imm2:
                np.maximum(in0, s0) - np.minimum(in1, s1))),
        # inter = relu(wd) * relu(hd)
        "BHM_RELUMUL": reg("BHM_RELUMUL", Spec(
            body=relu(Src0) * relu(Src1),
            reference=lambda in0, in1, s0, s1, imm2:
                np.maximum(in0, 0) * np.maximum(in1, 0))),
        # union = (a2_t * 1 + a1_q) - inter; generic 2-tensor FMA
        "BHM_FMA3": reg("BHM_FMA3", Spec(
            body=(Src0 * C2 + C0) + Src1 * C1,
            reference=lambda in0, in1, s0, s1, imm2:
                (in0 * imm2 + s0) + in1 * s1)),
        # N = union^2 + inter*area_e
        "BHM_SQADD": reg("BHM_SQADD", Spec(
            body=Src0 * Src0 + Src1,
            reference=lambda in0, in1, s0, s1, imm2: in0 * in0 + in1)),
        # gq = (N * r) * (-2)
        "BHM_MULSC": reg("BHM_MULSC", Spec(
            body=(Src0 * Src1) * C2,
            reference=lambda in0, in1, s0, s1, imm2: (in0 * in1) * imm2)),
        # a12 = |cx_t - cx_q| + |cy_t - cy_q|
        "BHM_ABS2": reg("BHM_ABS2", Spec(
            body=maxx(_d0, Zero - _d0) + maxx(_d1, Zero - _d1),
            reference=lambda in0, in1, s0, s1, imm2:
                np.abs(in0 - s0) + np.abs(in1 - s1))),
        # s = (a12 + a34) * 5 + cc2_q
        "BHM_SCOMB": reg("BHM_SCOMB", Spec(
            body=(Src0 + Src1) * C2 + C0,
            reference=lambda in0, in1, s0, s1, imm2: (in0 + in1) * imm2 + s0)),
        # C = mask ? cv : 1e9
        "BHM_MASKSEL": reg("BHM_MASKSEL", Spec(
            body=select(Src1, Src0, C2),
            reference=lambda in0, in1, s0, s1, imm2:
                np.where(in1 != 0, in0, imm2))),
    }
    return _OPS


def _plan(num_boxes):
    """Sort batches by num_boxes; slot j holds sorted[8j:8j+8] (one per core).
    Returns (order[Bslots][cores] batch indices, W[Bslots] column widths)."""
    nb = np.asarray(num_boxes).astype(np.int64)
    order = np.argsort(nb, kind="stable")
    slots = order.reshape(B_PER, N_CORES)        # slot j, core c -> batch id
    W = []
    for j in range(B_PER):
        w = int(nb[slots[j]].max())
        w = min(T, w + (w & 1))                  # even width
        W.append(w)
    return slots, tuple(W)


def _build_program(W):
    from contextlib import ExitStack

    import concourse.bass as bass
    from concourse import mybir

    ops = _get_ops()
    f32 = mybir.dt.float32
    aluop = mybir.AluOpType
    nc = bass.Bass("TRN2")

    trep = nc.dram_tensor("trep", [B_PER, NREP, 128, T], f32, kind="ExternalInput").ap()
    qsc = nc.dram_tensor("qsc", [B_PER, QT, NQT * NSC], f32, kind="ExternalInput").ap()
    cout = nc.dram_tensor("C", [B_PER, Q, T], f32, kind="ExternalOutput").ap()

    NCO = 4           # output buffer slots
    NT = B_PER * NQT  # 60 tiles
    REPEAT = int(os.environ.get("BHM_REPEAT", "1"))
    N_IN_DMAS = B_PER * NREP + B_PER
    WMAX = max(W)

    with ExitStack() as ctx:
        rep = {}
        for b in range(B_PER):
            for r in range(NREP):
                rep[(b, r)] = ctx.enter_context(
                    nc.sbuf_tensor(f"rep_{b}_{r}", [128, T], f32))
        qs = [ctx.enter_context(nc.sbuf_tensor(f"qs_{b}", [QT, NQT * NSC], f32))
              for b in range(B_PER)]
        # all work tensors double-buffered (two tiles in flight per level)
        names = ["wd", "hd", "we", "he", "inter", "areae", "union", "n1",
                 "nn", "dd", "rr", "gq", "a1", "a2", "a3", "a4",
                 "s1", "s2", "sp", "cv"]
        wkt = {n: [ctx.enter_context(
            nc.sbuf_tensor(f"wk_{n}_{s}", [QT, WMAX], f32)) for s in range(2)]
            for n in names}
        co = [ctx.enter_context(nc.sbuf_tensor(f"co_{i}", [QT, WMAX], f32))
              for i in range(NCO)]

        inb_sems = [ctx.enter_context(nc.semaphore(f"inb_sem_{b}"))
                    for b in range(B_PER)]
        fill_sem = ctx.enter_context(nc.semaphore("fill_sem"))
        act_sem = ctx.enter_context(nc.semaphore("act_sem"))
        g1_sem = ctx.enter_context(nc.semaphore("g1_sem"))
        dve_sem = ctx.enter_context(nc.semaphore("dve_sem"))
        out_sems = [ctx.enter_context(nc.semaphore(f"out_sem_{i}"))
                    for i in range(NCO)]
        block = ctx.enter_context(nc.Block())

        NK = NT * REPEAT

        def tile_ctx(k):
            b, qt = divmod(k % NT, NQT)
            n = W[b]

            def rp(r):
                return rep[(b, r)][0:QT, 0:n]

            def sc(s):
                return qs[b][:, qt * NSC + s:qt * NSC + s + 1]

            def w(nm, slot=0):
                return wkt[nm][slot][:, 0:n]

            return b, n, rp, sc, w

        @block.sync
        def _(sync):
            for b in range(B_PER):
                for r in range(NREP):
                    sync.dma_start(out=rep[(b, r)][:], in_=trep[b, r]).then_inc(inb_sems[b], 16)
                sync.dma_start(out=qs[b][:], in_=qsc[b]).then_inc(inb_sems[b], 16)
            for k in range(NT * REPEAT):
                b, qt = divmod(k % NT, NQT)
                n = W[b]
                q0 = min(qt * QT, Q - QT)
                r0 = QT - 8 if qt == NQT - 1 else 0
                sync.wait_ge(dve_sem, k + 1)
                sync.dma_start(out=cout[b, q0 + r0:q0 + QT, 0:n],
                               in_=co[k % NCO][r0:QT, 0:n]).then_inc(out_sems[k % NCO], 16)

        # ACT: the four L1 |coord_t - coord_q| terms (free per-partition
        # bias). The constant-1e9 fill DMAs also issue from ACT's queue
        # (it has slack) so they never delay the sync queue's output stores.
        @block.scalar
        def _(a):
            for b in range(B_PER):
                if W[b] < T:
                    fw = T - W[b]
                    a.wait_ge(inb_sems[b], 16 * (NREP + 1))
                    for qt in range(NQT):
                        q0 = min(qt * QT, Q - QT)
                        r0 = QT - 8 if qt == NQT - 1 else 0
                        a.dma_start(
                            out=cout[b, q0 + r0:q0 + QT, W[b]:T],
                            in_=rep[(b, R_FILL)][r0:QT, 0:fw],
                        ).then_inc(fill_sem, 16)
            loaded_b = -1
            for k in range(NK):
                b, n, rp, sc, w = tile_ctx(k)
                bb = (k % NT) // NQT
                while loaded_b < bb:
                    loaded_b += 1
                    a.wait_ge(inb_sems[loaded_b], 16 * (NREP + 1))
                if k >= 2:
                    a.wait_ge(g1_sem, k - 1)   # a slots free after gpsimd sums
                s = k % 2
                a.activation(w("a1", s), rp(R_CX), mybir.ActivationFunctionType.Abs,
                             bias=sc(S_NCX))
                a.activation(w("a2", s), rp(R_CY), mybir.ActivationFunctionType.Abs,
                             bias=sc(S_NCY))
                a.activation(w("a3", s), rp(R_W), mybir.ActivationFunctionType.Abs,
                             bias=sc(S_NW))
                a.activation(w("a4", s), rp(R_H), mybir.ActivationFunctionType.Abs,
                             bias=sc(S_NH))
                a.drain().then_inc(act_sem, 1)

        # GPSIMD: sum the L1 terms and pre-scale:  sp = (a1+a2+a3+a4) + cc2/5
        @block.gpsimd
        def _(g):
            for k in range(NK):
                b, n, rp, sc, w = tile_ctx(k)
                s = k % 2
                g.wait_ge(act_sem, k + 1)
                if k >= 2:
                    g.wait_ge(dve_sem, k - 1)  # sp slot read by cv(k-2)
                g.tensor_tensor(w("s1", s), w("a1", s), w("a2", s), op=aluop.add)
                g.tensor_tensor(w("s2", s), w("a3", s), w("a4", s), op=aluop.add)
                g.drain()
                g.tensor_tensor(w("sp", s), w("s1", s), w("s2", s), op=aluop.add)
                g.drain().then_inc(g1_sem, 1)

        # All compute on the DVE. Two tiles are processed per iteration with
        # shared per-level drains (halves the drain count; the two tiles'
        # ops within a level are independent). Input waits are staged per
        # batch slot so compute starts after the first slot's loads land.
        @block.vector
        def _(v):
            cd = v._custom_dve
            assert NK % 2 == 0
            loaded_b = -1
            for k0 in range(0, NK, 2):
                pair = [(k0, *tile_ctx(k0)), (k0 + 1, *tile_ctx(k0 + 1))]
                need_b = max((min(k, NK - 1) % NT) // NQT for k in (k0, k0 + 1))
                while loaded_b < need_b:
                    loaded_b += 1
                    v.wait_ge(inb_sems[loaded_b], 16 * (NREP + 1))

                for k, b, n, rp, sc, w in pair:      # L0
                    s = k % 2
                    cd(ops["BHM_IDIFF"], out=w("wd", s), in0=rp(R_X1),
                       in1=rp(R_X0), s0=sc(S_X1), s1=sc(S_X0))
                    cd(ops["BHM_IDIFF"], out=w("hd", s), in0=rp(R_Y1),
                       in1=rp(R_Y0), s0=sc(S_Y1), s1=sc(S_Y0))
                    cd(ops["BHM_EDIFF"], out=w("we", s), in0=rp(R_X1),
                       in1=rp(R_X0), s0=sc(S_X1), s1=sc(S_X0))
                    cd(ops["BHM_EDIFF"], out=w("he", s), in0=rp(R_Y1),
                       in1=rp(R_Y0), s0=sc(S_Y1), s1=sc(S_Y0))
                v.drain()
                for k, b, n, rp, sc, w in pair:      # L1
                    s = k % 2
                    cd(ops["BHM_RELUMUL"], out=w("inter", s), in0=w("wd", s),
                       in1=w("hd", s))
                    v.tensor_tensor(w("areae", s), w("we", s), w("he", s),
                                    op=aluop.mult)
                v.drain()
                for k, b, n, rp, sc, w in pair:      # L2
                    s = k % 2
                    cd(ops["BHM_FMA3"], out=w("union", s), in0=rp(R_A2),
                       in1=w("inter", s), s0=sc(S_A1), s1=-1.0, imm2=1.0)
                    v.tensor_tensor(w("n1", s), w("inter", s), w("areae", s),
                                    op=aluop.mult)
                v.drain()
                for k, b, n, rp, sc, w in pair:      # L3
                    s = k % 2
                    cd(ops["BHM_SQADD"], out=w("nn", s), in0=w("union", s),
                       in1=w("n1", s))
                    v.tensor_tensor(w("dd", s), w("union", s), w("areae", s),
                                    op=aluop.mult)
                v.drain()
                for k, b, n, rp, sc, w in pair:      # L4
                    s = k % 2
                    v.reciprocal_approx_fast(out=w("rr", s), in_=w("dd", s))
                v.drain()
                for k, b, n, rp, sc, w in pair:      # L5
                    s = k % 2
                    cd(ops["BHM_MULSC"], out=w("gq", s), in0=w("nn", s),
                       in1=w("rr", s), imm2=-2.0)
                v.drain()
                for k, b, n, rp, sc, w in pair:      # L6: cv = 5*sp + cc2 + gq
                    s = k % 2
                    v.wait_ge(g1_sem, k + 1)
                    cd(ops["BHM_FMA3"], out=w("cv", s), in0=w("sp", s),
                       in1=w("gq", s), s0=sc(S_CC), s1=1.0, imm2=5.0)
                v.drain()
                for k, b, n, rp, sc, w in pair:      # L7: mask + emit
                    if k >= NCO:
                        v.wait_ge(out_sems[k % NCO], 16 * (k // NCO))
                    cd(ops["BHM_MASKSEL"], out=co[k % NCO][:, 0:n],
                       in0=w("cv", k % 2), in1=rp(R_MK), imm2=INVALID)
                v.drain().then_inc(dve_sem, 1)
                v.drain().then_inc(dve_sem, 1)

    # Raw Bass skips Bacc.compile()'s codegen_inst_isa_subclasses pass;
    # without it InstCustomDveAnt .instr stays empty and walrus rejects the
    # NEFF with "ISA wrong length".
    mybir.codegen_inst_isa_subclasses(nc)
    return nc


def _host_prep(pred_logits, pred_boxes, boxes_padded, num_boxes, slots):
    """Per-core input maps (separable terms in fp64); slots[j][c] = batch id."""
    pl = np.asarray(pred_logits, np.float64)[..., 0]          # [B,Q]
    pb = np.asarray(pred_boxes, np.float64)                   # [B,Q,4]
    tb = np.asarray(boxes_padded, np.float64)                 # [B,T,4]
    nb = np.asarray(num_boxes).astype(np.int64)               # [B]

    cx, cy, w, h = pb[..., 0], pb[..., 1], pb[..., 2], pb[..., 3]
    x0q, y0q = cx - 0.5 * w, cy - 0.5 * h
    x1q, y1q = cx + 0.5 * w, cy + 0.5 * h
    area1 = (x1q - x0q) * (y1q - y0q)

    p = 1.0 / (1.0 + np.exp(-pl))
    log_p = -np.log1p(np.exp(-pl))
    log_1mp = -np.log1p(np.exp(pl))
    cc = -0.25 * (1.0 - p) ** 2 * log_p + 0.75 * p ** 2 * log_1mp
    cc2 = 2.0 * cc + 2.0

    tcx, tcy, tw, th = tb[..., 0], tb[..., 1], tb[..., 2], tb[..., 3]
    tx0, ty0 = tcx - 0.5 * tw, tcy - 0.5 * th
    tx1, ty1 = tcx + 0.5 * tw, tcy + 0.5 * th
    area2 = (tx1 - tx0) * (ty1 - ty0)
    mk = (np.arange(T)[None, :] < nb[:, None]).astype(np.float64)   # [B,T]
    fill = np.full((B, T), INVALID, np.float64)

    trows = np.stack([tx0, ty0, tx1, ty1, tcx, tcy, tw, th, area2, mk, fill],
                     axis=1)                                   # [B,NREP,T]
    qcols = np.stack([x0q, y0q, x1q, y1q, cx, cy, w, h, area1, cc2,
                      -cx, -cy, -w, -h, cc2 / 5.0],
                     axis=2)                                   # [B,Q,NSC]

    in_maps = []
    for c in range(N_CORES):
        bs = [int(slots[j][c]) for j in range(B_PER)]
        trep = np.broadcast_to(trows[bs][:, :, None, :], (B_PER, NREP, 128, T))
        trep = np.ascontiguousarray(trep, dtype=np.float32)
        # tile qt covers q rows [q0, q0+128), last tile overlapping
        qb = qcols[bs]
        tiles = [qb[:, min(qt * QT, Q - QT):min(qt * QT, Q - QT) + QT, :]
                 for qt in range(NQT)]
        qc = np.stack(tiles, axis=1)                  # [B_PER, NQT, QT, NSC]
        qc = qc.transpose(0, 2, 1, 3)
        qc = np.ascontiguousarray(qc.reshape(B_PER, QT, NQT * NSC),
                                  dtype=np.float32)
        in_maps.append({"trep": trep, "qsc": qc})
    return in_maps


def kernel(pred_logits, pred_boxes, boxes_padded, num_boxes):
    global LAST_RESULTS
    from concourse.bass_utils import run_bass_kernel_spmd

    slots, W = _plan(num_boxes)
    in_maps = _host_prep(pred_logits, pred_boxes, boxes_padded, num_boxes, slots)
    nc = _PROG_CACHE.get(W)
    if nc is None:
        nc = _build_program(W)
        _PROG_CACHE[W] = nc
    trace = bool(int(os.environ.get("BHM_TRACE", "0")))
    res = run_bass_kernel_spmd(nc, in_maps, list(range(N_CORES)), trace=trace)
    LAST_RESULTS = res
    out = np.empty((B, Q, T), np.float32)
    for c in range(N_CORES):
        slab = np.asarray(res.results[c]["C"]).reshape(B_PER, Q, T)
        for j in range(B_PER):
            out[int(slots[j][c])] = slab[j]
    return out
